# revision 12
# baseline (speedup 1.0000x reference)
"""Trainium2 Bass kernel for nn_ContrastiveLoss (data-parallel over 8 NeuronCores).

Computation (per row b of B=65536):
    ed[b]   = (sum_k ||o[b]-v[k,b]+eps||_2 + alpha*||o[b]-f[b]+eps||_2) / sqrt(D)
    loss[b] = (1-lab)*ed^2/2 + lab*relu((K+alpha)*0.8 - ed)^2/2

Strategy: pure data-parallel row sharding, 8192 rows per core.  Inputs are
downcast to fp16 on the host (loss rel-err ~1e-4, HBM traffic halved; the
problem is memory-regime).  Per core the rows are laid out as
[128 partitions x 64 row-tiles] with row = p*64 + t so every DMA is
contiguous-per-partition.  Row-tiles are processed in groups of G=4; the
9 squared-distance reductions per row are split across engines:
  - 4 of 9 through ScalarE: DVE tensor_tensor subtract (fp16 2x mode, one
    [P, G*D] pass), then 4x activation(Square, bias=eps, accum_out=col).
  - 5 of 9 through a custom fused DVE scan op: out = running prefix sum of
    ((o - v) + eps)^2 over the whole [P, G*D] stream in ONE 2-port pass;
    the 4 per-tile sums are recovered from the segment-boundary cumsums by
    one small strided copy per group plus a single differencing pass at
    the end.  (Registered into dve_ops.OPS at import time.)
The cheap per-row tail (sqrt, hinge, blend by label) runs once per core on
[128, 64] tiles.
"""

import os
import sys

import numpy as np

for _p in ("/root/.axon_site/_ro/trn_rl_repo", "/opt/trn_rl_repo"):
    if os.path.isdir(_p) and _p not in sys.path:
        sys.path.append(_p)

import concourse.bass as bass
import concourse.bacc as bacc
import concourse.dve_ops as dve_ops_mod
import concourse.mybir as mybir
from concourse import tile
from concourse.bass_utils import run_bass_kernel_spmd
from concourse.dve_spec import C0, C1, AluOp, Spec, Src0, Src1, lower, scan, sq
from concourse.dve_uop import DveOpSpec

B, D, K = 65536, 512, 8
M = 8                      # cores
BC = B // M                # rows per core
P = 128                    # SBUF partitions
T = BC // P                # row-tiles per core (64)
G = 4                      # row-tiles per load group
NG = T // G
KP1 = K + 1
NC_COLS = T * KP1          # sumsq columns per core (576)
EPS = 1e-6
MARGIN = 0.8

# engine split: these k's go through ACT (k == K is the feat1 distance);
# the rest (a contiguous range) go through the fused DVE scan op.  The split
# alternates 4/5 between even and odd groups so DVE and ACT average out at
# 4.5 reductions each.
ACT_KS_EVEN = (0, 1, 2, K)          # customs 3..7
ACT_KS_ODD = (0, 1, 2, 3, K)        # customs 4..7

_F32 = mybir.dt.float32
_F16 = mybir.dt.float16
_I32 = mybir.dt.int32

_prog_cache: dict = {}


def _register_op(name, spec):
    for op in dve_ops_mod.OPS:
        if op.name == name:
            return op
    row = dve_ops_mod._CUSTOM_DVE_ROW_BASE + len(dve_ops_mod.OPS)
    assert row < 0x20
    op = dve_ops_mod.DveOp(name, spec, subdim=False, uops_sha={})
    dve_ops_mod.OPS.append(op)
    dve_ops_mod.CUSTOM_DVE_SPECS[name] = spec
    dve_ops_mod._SUB_OPCODE_FOR_NAME[name] = row
    # pin the sha exactly as DveOp.compile() derives it
    for ver in ("v3",):
        s = DveOpSpec(name=name, opcode=row, uops=lower(spec, ver=ver), rd1_en=True)
        op.uops_sha[ver] = s.sha(ver)
    return op


def _subsq_scan_ref(in0, in1, c0, c1, c2):
    b = (((in0.astype(np.float32) - in1.astype(np.float32)) + c1) ** 2).astype(
        np.float32
    )
    pdim = b.shape[0]
    return np.cumsum(b.reshape(pdim, -1), axis=1, dtype=np.float32).reshape(b.shape)


# out = prefix-sum of ((in0 - in1) + c1)^2 along the free stream
SUBSQ_SCAN = _register_op(
    "SUBSQ_SCAN_ANT",
    Spec(
        body=scan(AluOp.ADD, sq((Src0 - Src1) + C1)),
        reference=_subsq_scan_ref,
    ),
)


def _build_program(alpha: float) -> bass.Bass:
    from contextlib import ExitStack

    nc = bacc.Bacc("TRN2", target_bir_lowering=False, debug=False)

    o_d = nc.dram_tensor("output1", [BC, D], _F16, kind="ExternalInput")
    v_d = nc.dram_tensor("vectors", [K, BC, D], _F16, kind="ExternalInput")
    f_d = nc.dram_tensor("feat1", [BC, D], _F16, kind="ExternalInput")
    l_d = nc.dram_tensor("label", [BC], _I32, kind="ExternalInput")
    out_d = nc.dram_tensor("loss", [BC], _F32, kind="ExternalOutput")

    # Row r of this core lives at (partition p, tile t) with r = p*T + t, so
    # each partition's 64 rows are contiguous in DRAM.
    o2 = o_d.ap().rearrange("(p t) d -> p (t d)", p=P)
    f2 = f_d.ap().rearrange("(p t) d -> p (t d)", p=P)
    v2 = v_d.ap().rearrange("k (p t) d -> k p (t d)", p=P)
    l2 = l_d.ap().rearrange("(p t) -> p t", p=P)
    out2 = out_d.ap().rearrange("(p t) -> p t", p=P)

    FD = G * D
    inv_sqrt_d = 1.0 / float(np.sqrt(np.float32(D)))
    c1 = float(np.sqrt(0.5)) * inv_sqrt_d
    marg = (K + alpha) * MARGIN
    r_bias = float(np.sqrt(0.5)) * marg

    SQUARE = mybir.ActivationFunctionType.Square
    SQRT = mybir.ActivationFunctionType.Sqrt
    RELU = mybir.ActivationFunctionType.Relu
    SUB = mybir.AluOpType.subtract

    # interleave ACT-path and DVE-path k's so both engines stay fed
    k_order_even = [0, 3, 1, 4, 2, 5, K, 6, 7]
    k_order_odd = [0, 4, 1, 5, 2, 6, 3, 7, K]

    with tile.TileContext(nc) as tc, ExitStack() as ctx:
        io_pool = ctx.enter_context(tc.tile_pool(name="io", bufs=3))
        v_pool = ctx.enter_context(tc.tile_pool(name="vp", bufs=18))
        diff_pool = ctx.enter_context(tc.tile_pool(name="dp", bufs=4))
        scan_pool = ctx.enter_context(tc.tile_pool(name="cp", bufs=3))
        sq_pool = ctx.enter_context(tc.tile_pool(name="sp", bufs=6))
        acc_pool = ctx.enter_context(tc.tile_pool(name="ap", bufs=1))
        tail_pool = ctx.enter_context(tc.tile_pool(name="tp", bufs=1))

        sumsq = acc_pool.tile([P, NC_COLS], _F32, name="sumsq")
        cs_sq = acc_pool.tile([P, NC_COLS], _F32, name="cs_sq")
        eps_b = tail_pool.tile([P, 1], _F32, name="eps_b")
        nc.vector.memset(eps_b[:], EPS)

        for g in range(NG):
            act_ks = ACT_KS_EVEN if g % 2 == 0 else ACT_KS_ODD
            k_order = k_order_even if g % 2 == 0 else k_order_odd
            t0 = g * G
            sl = slice(t0 * D, (t0 + G) * D)
            o_t = io_pool.tile([P, FD], _F16, tag="o", name=f"o_{g}")
            nc.sync.dma_start(o_t[:], o2[:, sl])
            f_t = io_pool.tile([P, FD], _F16, tag="f", name=f"f_{g}")
            nc.sync.dma_start(f_t[:], f2[:, sl])
            for k in k_order:
                if k < K:
                    x_t = v_pool.tile([P, FD], _F16, tag="v", name=f"v_{g}_{k}")
                    nc.sync.dma_start(x_t[:], v2[k, :, sl])
                else:
                    x_t = f_t
                if k in act_ks:
                    # DVE 2x fp16 subtract, then ACT square+accum (bias=eps)
                    diff = diff_pool.tile([P, FD], _F16, tag="d", name=f"df_{g}_{k}")
                    nc.vector.tensor_tensor(
                        out=diff[:], in0=o_t[:], in1=x_t[:], op=SUB,
                    )
                    for j in range(G):
                        sqt = sq_pool.tile([P, D], _F16, tag="s", name=f"sq_{g}_{k}_{j}")
                        col = (t0 + j) * KP1 + k
                        nc.scalar.activation(
                            sqt[:], diff[:, j * D:(j + 1) * D], SQUARE,
                            bias=eps_b[:],
                            accum_out=sumsq[:, col:col + 1],
                        )
                else:
                    # one fused DVE scan over the whole [P, G*D] stream, then
                    # pull the G segment-boundary cumsums into cs_sq columns
                    scan_t = scan_pool.tile([P, FD], _F32, tag="z", name=f"zs_{g}_{k}")
                    nc.vector._custom_dve(
                        SUBSQ_SCAN,
                        out=scan_t[:],
                        in0=o_t[:],
                        in1=x_t[:],
                        s1=EPS,
                    )
                    c0 = t0 * KP1 + k
                    nc.gpsimd.tensor_copy(
                        cs_sq[:, c0:c0 + (G - 1) * KP1 + 1:KP1],
                        scan_t[:, D - 1::D],
                    )

        # recover per-tile sums from the cumsums:
        #   sumsq[t0+0] = cs[t0+0];  sumsq[t0+j] = cs[t0+j] - cs[t0+j-1]
        cs4 = cs_sq.rearrange("p (g j n) -> p g j n", g=NG, j=G)
        ss4 = sumsq.rearrange("p (g j n) -> p g j n", g=NG, j=G)
        # customs 4..7 exist in every group; custom 3 only in even groups
        nc.vector.tensor_copy(ss4[:, :, 0:1, 4:8], cs4[:, :, 0:1, 4:8])
        nc.vector.tensor_sub(
            ss4[:, :, 1:G, 4:8], cs4[:, :, 1:G, 4:8], cs4[:, :, 0:G - 1, 4:8],
        )
        nc.vector.tensor_copy(ss4[:, 0:NG:2, 0:1, 3:4], cs4[:, 0:NG:2, 0:1, 3:4])
        nc.vector.tensor_sub(
            ss4[:, 0:NG:2, 1:G, 3:4],
            cs4[:, 0:NG:2, 1:G, 3:4],
            cs4[:, 0:NG:2, 0:G - 1, 3:4],
        )

        # --- per-core tail on [P, T] ---
        d_all = tail_pool.tile([P, NC_COLS], _F32, name="d_all")
        if alpha == 1.0:
            nc.scalar.activation(d_all[:], sumsq[:], SQRT)
        else:
            s3 = sumsq.rearrange("p (t n) -> p t n", n=KP1)
            d3 = d_all.rearrange("p (t n) -> p t n", n=KP1)
            nc.scalar.activation(d3[:, :, 0:K], s3[:, :, 0:K], SQRT)
            # alpha * sqrt(x) == sqrt(alpha^2 * x)
            nc.scalar.activation(
                d3[:, :, K:KP1], s3[:, :, K:KP1], SQRT,
                scale=float(alpha) * float(alpha),
            )
        s_t = tail_pool.tile([P, T], _F32, name="s_t")
        nc.vector.reduce_sum(
            s_t[:], d_all.rearrange("p (t n) -> p t n", n=KP1),
            axis=mybir.AxisListType.X,
        )
        # neg2 = 0.5*ed^2 = (c1*S)^2 ; pos2 = 0.5*relu(marg-ed)^2
        neg2 = tail_pool.tile([P, T], _F32, name="neg2")
        nc.scalar.activation(neg2[:], s_t[:], SQUARE, scale=c1)
        bias_t = tail_pool.tile([P, 1], _F32, name="bias_t")
        nc.vector.memset(bias_t[:], r_bias)
        r_t = tail_pool.tile([P, T], _F32, name="r_t")
        nc.scalar.activation(r_t[:], s_t[:], RELU, bias=bias_t[:], scale=-c1)
        pos2 = tail_pool.tile([P, T], _F32, name="pos2")
        nc.scalar.activation(pos2[:], r_t[:], SQUARE)

        lab_t = tail_pool.tile([P, T], _I32, name="lab_t")
        nc.sync.dma_start(lab_t[:], l2[:])
        lab_f = tail_pool.tile([P, T], _F32, name="lab_f")
        nc.vector.tensor_copy(lab_f[:], lab_t[:])

        # loss = neg2 + lab*(pos2 - neg2)
        dls = tail_pool.tile([P, T], _F32, name="dls")
        nc.vector.tensor_sub(dls[:], pos2[:], neg2[:])
        m_t = tail_pool.tile([P, T], _F32, name="m_t")
        nc.vector.tensor_mul(m_t[:], lab_f[:], dls[:])
        loss_t = tail_pool.tile([P, T], _F32, name="loss_t")
        nc.vector.tensor_add(loss_t[:], neg2[:], m_t[:])
        nc.sync.dma_start(out2[:], loss_t[:])

    nc.compile()
    return nc


def _get_program(alpha: float) -> bass.Bass:
    key = float(alpha)
    if key not in _prog_cache:
        _prog_cache[key] = _build_program(key)
    return _prog_cache[key]


def kernel(output1, vectors, feat1, label, alpha):
    output1 = np.asarray(output1, dtype=np.float32)
    vectors = np.asarray(vectors, dtype=np.float32)
    feat1 = np.asarray(feat1, dtype=np.float32)
    label = np.asarray(label, dtype=np.int32)
    alpha_f = float(np.asarray(alpha))

    nc = _get_program(alpha_f)

    in_maps = []
    for c in range(M):
        sl = slice(c * BC, (c + 1) * BC)
        in_maps.append({
            "output1": output1[sl].astype(np.float16),
            "vectors": vectors[:, sl].astype(np.float16),
            "feat1": feat1[sl].astype(np.float16),
            "label": np.ascontiguousarray(label[sl]),
        })

    trace = bool(int(os.environ.get("KERNEL_BASS_TRACE", "0")))
    if trace:
        try:
            res = run_bass_kernel_spmd(nc, in_maps, list(range(M)), trace=True)
        except Exception as e:
            print(f"trace run failed ({e!r}); rerunning without trace")
            res = run_bass_kernel_spmd(nc, in_maps, list(range(M)), trace=False)
        kernel.last_results = res
        if res.exec_time_ns is not None:
            print(f"HW exec time: {res.exec_time_ns} ns")
    else:
        res = run_bass_kernel_spmd(nc, in_maps, list(range(M)), trace=False)

    out = np.concatenate([res.results[c]["loss"] for c in range(M)])
    return np.ascontiguousarray(out.astype(np.float32, copy=False))


if __name__ == "__main__":
    rng = np.random.default_rng(0)
    o = rng.standard_normal((B, D), dtype=np.float32)
    v = rng.standard_normal((K, B, D), dtype=np.float32)
    f = rng.standard_normal((B, D), dtype=np.float32)
    lab = rng.integers(0, 2, size=(B,)).astype(np.int32)
    out = kernel(output1=o, vectors=v, feat1=f, label=lab, alpha=1)
    print(out.shape, out.dtype, out[:8])


# revision 13
# speedup vs baseline: 1.0230x; 1.0230x over previous
"""Trainium2 Bass kernel for nn_ContrastiveLoss (data-parallel over 8 NeuronCores).

Computation (per row b of B=65536):
    ed[b]   = (sum_k ||o[b]-v[k,b]+eps||_2 + alpha*||o[b]-f[b]+eps||_2) / sqrt(D)
    loss[b] = (1-lab)*ed^2/2 + lab*relu((K+alpha)*0.8 - ed)^2/2

Strategy: pure data-parallel row sharding, 8192 rows per core.  Inputs are
downcast to fp16 on the host (loss rel-err ~1e-4, HBM traffic halved; the
problem is memory-regime).  Per core the rows are laid out as
[128 partitions x 64 row-tiles] with row = p*64 + t so every DMA is
contiguous-per-partition.  Row-tiles are processed in groups of G=4; the
9 squared-distance reductions per row are split across engines:
  - 4 of 9 through ScalarE: DVE tensor_tensor subtract (fp16 2x mode, one
    [P, G*D] pass), then 4x activation(Square, bias=eps, accum_out=col).
  - 5 of 9 through a custom fused DVE scan op: out = running prefix sum of
    ((o - v) + eps)^2 over the whole [P, G*D] stream in ONE 2-port pass;
    the 4 per-tile sums are recovered from the segment-boundary cumsums by
    one small strided copy per group plus a single differencing pass at
    the end.  (Registered into dve_ops.OPS at import time.)
The cheap per-row tail (sqrt, hinge, blend by label) runs once per core on
[128, 64] tiles.
"""

import os
import sys

import numpy as np

for _p in ("/root/.axon_site/_ro/trn_rl_repo", "/opt/trn_rl_repo"):
    if os.path.isdir(_p) and _p not in sys.path:
        sys.path.append(_p)

import concourse.bass as bass
import concourse.bacc as bacc
import concourse.dve_ops as dve_ops_mod
import concourse.mybir as mybir
from concourse import tile
from concourse.bass_utils import run_bass_kernel_spmd
from concourse.dve_spec import C0, C1, AluOp, Spec, Src0, Src1, lower, scan, sq
from concourse.dve_uop import DveOpSpec

B, D, K = 65536, 512, 8
M = 8                      # cores
BC = B // M                # rows per core
P = 128                    # SBUF partitions
T = BC // P                # row-tiles per core (64)
G = 4                      # row-tiles per load group
NG = T // G
KP1 = K + 1
NC_COLS = T * KP1          # sumsq columns per core (576)
EPS = 1e-6
MARGIN = 0.8

# engine split: these k's go through ACT (k == K is the feat1 distance);
# the rest (must be a contiguous range) go through the fused DVE scan op.
ACT_KS = (0, 1, 2, K)
CUS_KS = tuple(k for k in range(KP1) if k not in ACT_KS)
assert CUS_KS == tuple(range(CUS_KS[0], CUS_KS[-1] + 1))
CK0, CK1 = CUS_KS[0], CUS_KS[-1] + 1

_F32 = mybir.dt.float32
_F16 = mybir.dt.float16
_I32 = mybir.dt.int32

_prog_cache: dict = {}


def _register_op(name, spec):
    for op in dve_ops_mod.OPS:
        if op.name == name:
            return op
    row = dve_ops_mod._CUSTOM_DVE_ROW_BASE + len(dve_ops_mod.OPS)
    assert row < 0x20
    op = dve_ops_mod.DveOp(name, spec, subdim=False, uops_sha={})
    dve_ops_mod.OPS.append(op)
    dve_ops_mod.CUSTOM_DVE_SPECS[name] = spec
    dve_ops_mod._SUB_OPCODE_FOR_NAME[name] = row
    # pin the sha exactly as DveOp.compile() derives it
    for ver in ("v3",):
        s = DveOpSpec(name=name, opcode=row, uops=lower(spec, ver=ver), rd1_en=True)
        op.uops_sha[ver] = s.sha(ver)
    return op


def _subsq_scan_ref(in0, in1, c0, c1, c2):
    b = (((in0.astype(np.float32) - in1.astype(np.float32)) + c1) ** 2).astype(
        np.float32
    )
    pdim = b.shape[0]
    return np.cumsum(b.reshape(pdim, -1), axis=1, dtype=np.float32).reshape(b.shape)


# out = prefix-sum of ((in0 - in1) + c1)^2 along the free stream
SUBSQ_SCAN = _register_op(
    "SUBSQ_SCAN_ANT",
    Spec(
        body=scan(AluOp.ADD, sq((Src0 - Src1) + C1)),
        reference=_subsq_scan_ref,
    ),
)


def _build_program(alpha: float) -> bass.Bass:
    from contextlib import ExitStack

    nc = bacc.Bacc("TRN2", target_bir_lowering=False, debug=False)

    o_d = nc.dram_tensor("output1", [BC, D], _F16, kind="ExternalInput")
    v_d = nc.dram_tensor("vectors", [K, BC, D], _F16, kind="ExternalInput")
    f_d = nc.dram_tensor("feat1", [BC, D], _F16, kind="ExternalInput")
    l_d = nc.dram_tensor("label", [BC], _I32, kind="ExternalInput")
    out_d = nc.dram_tensor("loss", [BC], _F32, kind="ExternalOutput")

    # Row r of this core lives at (partition p, tile t) with r = p*T + t, so
    # each partition's 64 rows are contiguous in DRAM.
    o2 = o_d.ap().rearrange("(p t) d -> p (t d)", p=P)
    f2 = f_d.ap().rearrange("(p t) d -> p (t d)", p=P)
    v2 = v_d.ap().rearrange("k (p t) d -> k p (t d)", p=P)
    l2 = l_d.ap().rearrange("(p t) -> p t", p=P)
    out2 = out_d.ap().rearrange("(p t) -> p t", p=P)

    FD = G * D
    inv_sqrt_d = 1.0 / float(np.sqrt(np.float32(D)))
    c1 = float(np.sqrt(0.5)) * inv_sqrt_d
    marg = (K + alpha) * MARGIN
    r_bias = float(np.sqrt(0.5)) * marg

    SQUARE = mybir.ActivationFunctionType.Square
    SQRT = mybir.ActivationFunctionType.Sqrt
    RELU = mybir.ActivationFunctionType.Relu
    SUB = mybir.AluOpType.subtract

    # interleave ACT-path and DVE-path k's so both engines stay fed
    k_order = [0, 3, 1, 4, 2, 5, K, 6, 7]

    with tile.TileContext(nc) as tc, ExitStack() as ctx:
        io_pool = ctx.enter_context(tc.tile_pool(name="io", bufs=3))
        v_pool = ctx.enter_context(tc.tile_pool(name="vp", bufs=18))
        diff_pool = ctx.enter_context(tc.tile_pool(name="dp", bufs=4))
        scan_pool = ctx.enter_context(tc.tile_pool(name="cp", bufs=3))
        sq_pool = ctx.enter_context(tc.tile_pool(name="sp", bufs=6))
        acc_pool = ctx.enter_context(tc.tile_pool(name="ap", bufs=1))
        tail_pool = ctx.enter_context(tc.tile_pool(name="tp", bufs=1))

        sumsq = acc_pool.tile([P, NC_COLS], _F32, name="sumsq")
        cs_sq = acc_pool.tile([P, NC_COLS], _F32, name="cs_sq")
        eps_b = tail_pool.tile([P, 1], _F32, name="eps_b")
        nc.vector.memset(eps_b[:], EPS)

        for g in range(NG):
            t0 = g * G
            sl = slice(t0 * D, (t0 + G) * D)
            o_t = io_pool.tile([P, FD], _F16, tag="o", name=f"o_{g}")
            nc.sync.dma_start(o_t[:], o2[:, sl])
            f_t = io_pool.tile([P, FD], _F16, tag="f", name=f"f_{g}")
            nc.sync.dma_start(f_t[:], f2[:, sl])
            for k in k_order:
                if k < K:
                    x_t = v_pool.tile([P, FD], _F16, tag="v", name=f"v_{g}_{k}")
                    nc.sync.dma_start(x_t[:], v2[k, :, sl])
                else:
                    x_t = f_t
                if k in ACT_KS:
                    # DVE 2x fp16 subtract, then ACT square+accum (bias=eps)
                    diff = diff_pool.tile([P, FD], _F16, tag="d", name=f"df_{g}_{k}")
                    nc.vector.tensor_tensor(
                        out=diff[:], in0=o_t[:], in1=x_t[:], op=SUB,
                    )
                    for j in range(G):
                        sqt = sq_pool.tile([P, D], _F16, tag="s", name=f"sq_{g}_{k}_{j}")
                        col = (t0 + j) * KP1 + k
                        nc.scalar.activation(
                            sqt[:], diff[:, j * D:(j + 1) * D], SQUARE,
                            bias=eps_b[:],
                            accum_out=sumsq[:, col:col + 1],
                        )
                else:
                    # one fused DVE scan over the whole [P, G*D] stream, then
                    # pull the G segment-boundary cumsums into cs_sq columns
                    scan_t = scan_pool.tile([P, FD], _F32, tag="z", name=f"zs_{g}_{k}")
                    nc.vector._custom_dve(
                        SUBSQ_SCAN,
                        out=scan_t[:],
                        in0=o_t[:],
                        in1=x_t[:],
                        s1=EPS,
                    )
                    c0 = t0 * KP1 + k
                    nc.vector.tensor_copy(
                        cs_sq[:, c0:c0 + (G - 1) * KP1 + 1:KP1],
                        scan_t[:, D - 1::D],
                    )

        # recover per-tile sums from the cumsums:
        #   sumsq[t0+0] = cs[t0+0];  sumsq[t0+j] = cs[t0+j] - cs[t0+j-1]
        cs4 = cs_sq.rearrange("p (g j n) -> p g j n", g=NG, j=G)
        ss4 = sumsq.rearrange("p (g j n) -> p g j n", g=NG, j=G)
        nc.vector.tensor_copy(ss4[:, :, 0:1, CK0:CK1], cs4[:, :, 0:1, CK0:CK1])
        nc.vector.tensor_sub(
            ss4[:, :, 1:G, CK0:CK1],
            cs4[:, :, 1:G, CK0:CK1],
            cs4[:, :, 0:G - 1, CK0:CK1],
        )

        # --- per-core tail on [P, T] ---
        d_all = tail_pool.tile([P, NC_COLS], _F32, name="d_all")
        if alpha == 1.0:
            nc.scalar.activation(d_all[:], sumsq[:], SQRT)
        else:
            s3 = sumsq.rearrange("p (t n) -> p t n", n=KP1)
            d3 = d_all.rearrange("p (t n) -> p t n", n=KP1)
            nc.scalar.activation(d3[:, :, 0:K], s3[:, :, 0:K], SQRT)
            # alpha * sqrt(x) == sqrt(alpha^2 * x)
            nc.scalar.activation(
                d3[:, :, K:KP1], s3[:, :, K:KP1], SQRT,
                scale=float(alpha) * float(alpha),
            )
        s_t = tail_pool.tile([P, T], _F32, name="s_t")
        nc.vector.reduce_sum(
            s_t[:], d_all.rearrange("p (t n) -> p t n", n=KP1),
            axis=mybir.AxisListType.X,
        )
        # neg2 = 0.5*ed^2 = (c1*S)^2 ; pos2 = 0.5*relu(marg-ed)^2
        neg2 = tail_pool.tile([P, T], _F32, name="neg2")
        nc.scalar.activation(neg2[:], s_t[:], SQUARE, scale=c1)
        bias_t = tail_pool.tile([P, 1], _F32, name="bias_t")
        nc.vector.memset(bias_t[:], r_bias)
        r_t = tail_pool.tile([P, T], _F32, name="r_t")
        nc.scalar.activation(r_t[:], s_t[:], RELU, bias=bias_t[:], scale=-c1)
        pos2 = tail_pool.tile([P, T], _F32, name="pos2")
        nc.scalar.activation(pos2[:], r_t[:], SQUARE)

        lab_t = tail_pool.tile([P, T], _I32, name="lab_t")
        nc.sync.dma_start(lab_t[:], l2[:])
        lab_f = tail_pool.tile([P, T], _F32, name="lab_f")
        nc.vector.tensor_copy(lab_f[:], lab_t[:])

        # loss = neg2 + lab*(pos2 - neg2)
        dls = tail_pool.tile([P, T], _F32, name="dls")
        nc.vector.tensor_sub(dls[:], pos2[:], neg2[:])
        m_t = tail_pool.tile([P, T], _F32, name="m_t")
        nc.vector.tensor_mul(m_t[:], lab_f[:], dls[:])
        loss_t = tail_pool.tile([P, T], _F32, name="loss_t")
        nc.vector.tensor_add(loss_t[:], neg2[:], m_t[:])
        nc.sync.dma_start(out2[:], loss_t[:])

    nc.compile()
    return nc


def _get_program(alpha: float) -> bass.Bass:
    key = float(alpha)
    if key not in _prog_cache:
        _prog_cache[key] = _build_program(key)
    return _prog_cache[key]


def kernel(output1, vectors, feat1, label, alpha):
    output1 = np.asarray(output1, dtype=np.float32)
    vectors = np.asarray(vectors, dtype=np.float32)
    feat1 = np.asarray(feat1, dtype=np.float32)
    label = np.asarray(label, dtype=np.int32)
    alpha_f = float(np.asarray(alpha))

    nc = _get_program(alpha_f)

    in_maps = []
    for c in range(M):
        sl = slice(c * BC, (c + 1) * BC)
        in_maps.append({
            "output1": output1[sl].astype(np.float16),
            "vectors": vectors[:, sl].astype(np.float16),
            "feat1": feat1[sl].astype(np.float16),
            "label": np.ascontiguousarray(label[sl]),
        })

    trace = bool(int(os.environ.get("KERNEL_BASS_TRACE", "0")))
    if trace:
        try:
            res = run_bass_kernel_spmd(nc, in_maps, list(range(M)), trace=True)
        except Exception as e:
            print(f"trace run failed ({e!r}); rerunning without trace")
            res = run_bass_kernel_spmd(nc, in_maps, list(range(M)), trace=False)
        kernel.last_results = res
        if res.exec_time_ns is not None:
            print(f"HW exec time: {res.exec_time_ns} ns")
    else:
        res = run_bass_kernel_spmd(nc, in_maps, list(range(M)), trace=False)

    out = np.concatenate([res.results[c]["loss"] for c in range(M)])
    return np.ascontiguousarray(out.astype(np.float32, copy=False))


if __name__ == "__main__":
    rng = np.random.default_rng(0)
    o = rng.standard_normal((B, D), dtype=np.float32)
    v = rng.standard_normal((K, B, D), dtype=np.float32)
    f = rng.standard_normal((B, D), dtype=np.float32)
    lab = rng.integers(0, 2, size=(B,)).astype(np.int32)
    out = kernel(output1=o, vectors=v, feat1=f, label=lab, alpha=1)
    print(out.shape, out.dtype, out[:8])


# revision 14
# speedup vs baseline: 1.0473x; 1.0237x over previous
"""Trainium2 Bass kernel for nn_ContrastiveLoss (data-parallel over 8 NeuronCores).

Computation (per row b of B=65536):
    ed[b]   = (sum_k ||o[b]-v[k,b]+eps||_2 + alpha*||o[b]-f[b]+eps||_2) / sqrt(D)
    loss[b] = (1-lab)*ed^2/2 + lab*relu((K+alpha)*0.8 - ed)^2/2

Strategy: pure data-parallel row sharding, 8192 rows per core.  Inputs are
downcast to fp16 on the host (loss rel-err ~1e-4, HBM traffic halved; the
problem is memory-regime).  Per core the rows are laid out as
[128 partitions x 64 row-tiles] with row = p*64 + t so every DMA is
contiguous-per-partition.  Row-tiles are processed in groups of G=4; the
9 squared-distance reductions per row are split across engines:
  - 4 of 9 through ScalarE: DVE tensor_tensor subtract (fp16 2x mode, one
    [P, G*D] pass), then 4x activation(Square, bias=eps, accum_out=col).
  - 5 of 9 through a custom fused DVE scan op: out = running prefix sum of
    ((o - v) + eps)^2 over the whole [P, G*D] stream in ONE 2-port pass;
    the 4 per-tile sums are recovered from the segment-boundary cumsums by
    one small strided copy per group plus a single differencing pass at
    the end.  (Registered into dve_ops.OPS at import time.)
The cheap per-row tail (sqrt, hinge, blend by label) runs once per core on
[128, 64] tiles.
"""

import os
import sys

import numpy as np

for _p in ("/root/.axon_site/_ro/trn_rl_repo", "/opt/trn_rl_repo"):
    if os.path.isdir(_p) and _p not in sys.path:
        sys.path.append(_p)

import concourse.bass as bass
import concourse.bacc as bacc
import concourse.dve_ops as dve_ops_mod
import concourse.mybir as mybir
from concourse import tile
from concourse.bass_utils import run_bass_kernel_spmd
from concourse.dve_spec import C0, C1, AluOp, Spec, Src0, Src1, lower, scan, sq
from concourse.dve_uop import DveOpSpec

B, D, K = 65536, 512, 8
M = 8                      # cores
BC = B // M                # rows per core
P = 128                    # SBUF partitions
T = BC // P                # row-tiles per core (64)
G = 4                      # row-tiles per load group
NG = T // G
KP1 = K + 1
NC_COLS = T * KP1          # sumsq columns per core (576)
EPS = 1e-6
MARGIN = 0.8

# engine split: these k's go through ACT (k == K is the feat1 distance);
# the rest (a contiguous range) go through the fused DVE scan op.  The split
# alternates 4/5 between even and odd groups so DVE and ACT average 4.5
# reductions each (DVE was 94% busy vs ACT 78% at a fixed 4/5 split).
ACT_KS_EVEN = (0, 1, 2, K)          # customs 3..7
ACT_KS_ODD = (0, 1, 2, 3, K)        # customs 4..7

_F32 = mybir.dt.float32
_F16 = mybir.dt.float16
_I32 = mybir.dt.int32

_prog_cache: dict = {}


def _register_op(name, spec):
    for op in dve_ops_mod.OPS:
        if op.name == name:
            return op
    row = dve_ops_mod._CUSTOM_DVE_ROW_BASE + len(dve_ops_mod.OPS)
    assert row < 0x20
    op = dve_ops_mod.DveOp(name, spec, subdim=False, uops_sha={})
    dve_ops_mod.OPS.append(op)
    dve_ops_mod.CUSTOM_DVE_SPECS[name] = spec
    dve_ops_mod._SUB_OPCODE_FOR_NAME[name] = row
    # pin the sha exactly as DveOp.compile() derives it
    for ver in ("v3",):
        s = DveOpSpec(name=name, opcode=row, uops=lower(spec, ver=ver), rd1_en=True)
        op.uops_sha[ver] = s.sha(ver)
    return op


def _subsq_scan_ref(in0, in1, c0, c1, c2):
    b = (((in0.astype(np.float32) - in1.astype(np.float32)) + c1) ** 2).astype(
        np.float32
    )
    pdim = b.shape[0]
    return np.cumsum(b.reshape(pdim, -1), axis=1, dtype=np.float32).reshape(b.shape)


# out = prefix-sum of ((in0 - in1) + c1)^2 along the free stream
SUBSQ_SCAN = _register_op(
    "SUBSQ_SCAN_ANT",
    Spec(
        body=scan(AluOp.ADD, sq((Src0 - Src1) + C1)),
        reference=_subsq_scan_ref,
    ),
)


def _build_program(alpha: float) -> bass.Bass:
    from contextlib import ExitStack

    nc = bacc.Bacc("TRN2", target_bir_lowering=False, debug=False)

    o_d = nc.dram_tensor("output1", [BC, D], _F16, kind="ExternalInput")
    v_d = nc.dram_tensor("vectors", [K, BC, D], _F16, kind="ExternalInput")
    f_d = nc.dram_tensor("feat1", [BC, D], _F16, kind="ExternalInput")
    l_d = nc.dram_tensor("label", [BC], _I32, kind="ExternalInput")
    out_d = nc.dram_tensor("loss", [BC], _F32, kind="ExternalOutput")

    # Row r of this core lives at (partition p, tile t) with r = p*T + t, so
    # each partition's 64 rows are contiguous in DRAM.
    o2 = o_d.ap().rearrange("(p t) d -> p (t d)", p=P)
    f2 = f_d.ap().rearrange("(p t) d -> p (t d)", p=P)
    v2 = v_d.ap().rearrange("k (p t) d -> k p (t d)", p=P)
    l2 = l_d.ap().rearrange("(p t) -> p t", p=P)
    out2 = out_d.ap().rearrange("(p t) -> p t", p=P)

    FD = G * D
    inv_sqrt_d = 1.0 / float(np.sqrt(np.float32(D)))
    c1 = float(np.sqrt(0.5)) * inv_sqrt_d
    marg = (K + alpha) * MARGIN
    r_bias = float(np.sqrt(0.5)) * marg

    SQUARE = mybir.ActivationFunctionType.Square
    SQRT = mybir.ActivationFunctionType.Sqrt
    RELU = mybir.ActivationFunctionType.Relu
    SUB = mybir.AluOpType.subtract

    # interleave ACT-path and DVE-path k's so both engines stay fed
    k_order_even = [0, 3, 1, 4, 2, 5, K, 6, 7]
    k_order_odd = [0, 4, 1, 5, 2, 6, 3, 7, K]

    with tile.TileContext(nc) as tc, ExitStack() as ctx:
        io_pool = ctx.enter_context(tc.tile_pool(name="io", bufs=3))
        v_pool = ctx.enter_context(tc.tile_pool(name="vp", bufs=18))
        diff_pool = ctx.enter_context(tc.tile_pool(name="dp", bufs=4))
        scan_pool = ctx.enter_context(tc.tile_pool(name="cp", bufs=3))
        sq_pool = ctx.enter_context(tc.tile_pool(name="sp", bufs=6))
        acc_pool = ctx.enter_context(tc.tile_pool(name="ap", bufs=1))
        tail_pool = ctx.enter_context(tc.tile_pool(name="tp", bufs=1))

        sumsq = acc_pool.tile([P, NC_COLS], _F32, name="sumsq")
        cs_sq = acc_pool.tile([P, NC_COLS], _F32, name="cs_sq")
        eps_b = tail_pool.tile([P, 1], _F32, name="eps_b")
        nc.vector.memset(eps_b[:], EPS)

        for g in range(NG):
            act_ks = ACT_KS_EVEN if g % 2 == 0 else ACT_KS_ODD
            k_order = k_order_even if g % 2 == 0 else k_order_odd
            t0 = g * G
            sl = slice(t0 * D, (t0 + G) * D)
            o_t = io_pool.tile([P, FD], _F16, tag="o", name=f"o_{g}")
            nc.sync.dma_start(o_t[:], o2[:, sl])
            f_t = io_pool.tile([P, FD], _F16, tag="f", name=f"f_{g}")
            nc.sync.dma_start(f_t[:], f2[:, sl])
            for k in k_order:
                if k < K:
                    x_t = v_pool.tile([P, FD], _F16, tag="v", name=f"v_{g}_{k}")
                    nc.sync.dma_start(x_t[:], v2[k, :, sl])
                else:
                    x_t = f_t
                if k in act_ks:
                    # DVE 2x fp16 subtract, then ACT square+accum (bias=eps)
                    diff = diff_pool.tile([P, FD], _F16, tag="d", name=f"df_{g}_{k}")
                    nc.vector.tensor_tensor(
                        out=diff[:], in0=o_t[:], in1=x_t[:], op=SUB,
                    )
                    for j in range(G):
                        sqt = sq_pool.tile([P, D], _F16, tag="s", name=f"sq_{g}_{k}_{j}")
                        col = (t0 + j) * KP1 + k
                        nc.scalar.activation(
                            sqt[:], diff[:, j * D:(j + 1) * D], SQUARE,
                            bias=eps_b[:],
                            accum_out=sumsq[:, col:col + 1],
                        )
                else:
                    # one fused DVE scan over the whole [P, G*D] stream, then
                    # pull the G segment-boundary cumsums into cs_sq columns
                    scan_t = scan_pool.tile([P, FD], _F32, tag="z", name=f"zs_{g}_{k}")
                    nc.vector._custom_dve(
                        SUBSQ_SCAN,
                        out=scan_t[:],
                        in0=o_t[:],
                        in1=x_t[:],
                        s1=EPS,
                    )
                    c0 = t0 * KP1 + k
                    nc.vector.tensor_copy(
                        cs_sq[:, c0:c0 + (G - 1) * KP1 + 1:KP1],
                        scan_t[:, D - 1::D],
                    )

        # recover per-tile sums from the cumsums:
        #   sumsq[t0+0] = cs[t0+0];  sumsq[t0+j] = cs[t0+j] - cs[t0+j-1]
        cs4 = cs_sq.rearrange("p (g j n) -> p g j n", g=NG, j=G)
        ss4 = sumsq.rearrange("p (g j n) -> p g j n", g=NG, j=G)
        # customs 4..7 exist in every group; custom 3 only in even groups
        nc.vector.tensor_copy(ss4[:, :, 0:1, 4:8], cs4[:, :, 0:1, 4:8])
        nc.vector.tensor_sub(
            ss4[:, :, 1:G, 4:8], cs4[:, :, 1:G, 4:8], cs4[:, :, 0:G - 1, 4:8],
        )
        nc.vector.tensor_copy(ss4[:, 0:NG:2, 0:1, 3:4], cs4[:, 0:NG:2, 0:1, 3:4])
        nc.vector.tensor_sub(
            ss4[:, 0:NG:2, 1:G, 3:4],
            cs4[:, 0:NG:2, 1:G, 3:4],
            cs4[:, 0:NG:2, 0:G - 1, 3:4],
        )

        # --- per-core tail on [P, T] ---
        d_all = tail_pool.tile([P, NC_COLS], _F32, name="d_all")
        if alpha == 1.0:
            nc.scalar.activation(d_all[:], sumsq[:], SQRT)
        else:
            s3 = sumsq.rearrange("p (t n) -> p t n", n=KP1)
            d3 = d_all.rearrange("p (t n) -> p t n", n=KP1)
            nc.scalar.activation(d3[:, :, 0:K], s3[:, :, 0:K], SQRT)
            # alpha * sqrt(x) == sqrt(alpha^2 * x)
            nc.scalar.activation(
                d3[:, :, K:KP1], s3[:, :, K:KP1], SQRT,
                scale=float(alpha) * float(alpha),
            )
        s_t = tail_pool.tile([P, T], _F32, name="s_t")
        nc.vector.reduce_sum(
            s_t[:], d_all.rearrange("p (t n) -> p t n", n=KP1),
            axis=mybir.AxisListType.X,
        )
        # neg2 = 0.5*ed^2 = (c1*S)^2 ; pos2 = 0.5*relu(marg-ed)^2
        neg2 = tail_pool.tile([P, T], _F32, name="neg2")
        nc.scalar.activation(neg2[:], s_t[:], SQUARE, scale=c1)
        bias_t = tail_pool.tile([P, 1], _F32, name="bias_t")
        nc.vector.memset(bias_t[:], r_bias)
        r_t = tail_pool.tile([P, T], _F32, name="r_t")
        nc.scalar.activation(r_t[:], s_t[:], RELU, bias=bias_t[:], scale=-c1)
        pos2 = tail_pool.tile([P, T], _F32, name="pos2")
        nc.scalar.activation(pos2[:], r_t[:], SQUARE)

        lab_t = tail_pool.tile([P, T], _I32, name="lab_t")
        nc.sync.dma_start(lab_t[:], l2[:])
        lab_f = tail_pool.tile([P, T], _F32, name="lab_f")
        nc.vector.tensor_copy(lab_f[:], lab_t[:])

        # loss = neg2 + lab*(pos2 - neg2)
        dls = tail_pool.tile([P, T], _F32, name="dls")
        nc.vector.tensor_sub(dls[:], pos2[:], neg2[:])
        m_t = tail_pool.tile([P, T], _F32, name="m_t")
        nc.vector.tensor_mul(m_t[:], lab_f[:], dls[:])
        loss_t = tail_pool.tile([P, T], _F32, name="loss_t")
        nc.vector.tensor_add(loss_t[:], neg2[:], m_t[:])
        nc.sync.dma_start(out2[:], loss_t[:])

    nc.compile()
    return nc


def _get_program(alpha: float) -> bass.Bass:
    key = float(alpha)
    if key not in _prog_cache:
        _prog_cache[key] = _build_program(key)
    return _prog_cache[key]


def kernel(output1, vectors, feat1, label, alpha):
    output1 = np.asarray(output1, dtype=np.float32)
    vectors = np.asarray(vectors, dtype=np.float32)
    feat1 = np.asarray(feat1, dtype=np.float32)
    label = np.asarray(label, dtype=np.int32)
    alpha_f = float(np.asarray(alpha))

    nc = _get_program(alpha_f)

    in_maps = []
    for c in range(M):
        sl = slice(c * BC, (c + 1) * BC)
        in_maps.append({
            "output1": output1[sl].astype(np.float16),
            "vectors": vectors[:, sl].astype(np.float16),
            "feat1": feat1[sl].astype(np.float16),
            "label": np.ascontiguousarray(label[sl]),
        })

    trace = bool(int(os.environ.get("KERNEL_BASS_TRACE", "0")))
    if trace:
        try:
            res = run_bass_kernel_spmd(nc, in_maps, list(range(M)), trace=True)
        except Exception as e:
            print(f"trace run failed ({e!r}); rerunning without trace")
            res = run_bass_kernel_spmd(nc, in_maps, list(range(M)), trace=False)
        kernel.last_results = res
        if res.exec_time_ns is not None:
            print(f"HW exec time: {res.exec_time_ns} ns")
    else:
        res = run_bass_kernel_spmd(nc, in_maps, list(range(M)), trace=False)

    out = np.concatenate([res.results[c]["loss"] for c in range(M)])
    return np.ascontiguousarray(out.astype(np.float32, copy=False))


if __name__ == "__main__":
    rng = np.random.default_rng(0)
    o = rng.standard_normal((B, D), dtype=np.float32)
    v = rng.standard_normal((K, B, D), dtype=np.float32)
    f = rng.standard_normal((B, D), dtype=np.float32)
    lab = rng.integers(0, 2, size=(B,)).astype(np.int32)
    out = kernel(output1=o, vectors=v, feat1=f, label=lab, alpha=1)
    print(out.shape, out.dtype, out[:8])
